# revision 30
# baseline (speedup 1.0000x reference)
"""Trainium2 Bass kernel for nn_AttnBlock (GroupNorm + dense spatial attention).

Reference math (B=2, H=W=C=96, GROUPS=32, fp32):
    hn = GroupNorm32 over dim1(H) of x[B,H,W,C]  (stats over (3,W,C) per group)
    q/k/v = hn @ W* + b*;  scores = (q @ k^T)/sqrt(C) over HW=9216 per batch
    out = x + softmax(scores) @ v @ Wp + bp

Sharding (8 cores): core = (b, qc), b = core//4, qc = core%4. Each core holds
the full batch-b tensors (for K/V) plus its 2304-query-row chunk.

Design (v3 - fp8 DoubleRow PE + exp-balanced ACT/DVE):
  - The k-side tensor ships as RAW fp8e4 x in a channel-pair layout
    x8p[49, 2*HW] (channel c = i*49+p), normalized on-device into xsb8 by
    Pool (otherwise idle) from fp8 scale-row broadcasts.  Every contraction
    over the 97 aug channels then runs as a DoubleRow fp8 matmul at 0.5
    cycles/col - half the PE time of bf16 - for the score matmuls, the
    v-projection, and (existing key-pair trick) attn@V.
  - The k-side BIAS term of the scores is a per-query additive constant,
    softmax-invariant -> dropped (no ones row; pair slot 48/h1 is zero).
    bv is attention-invariant (sum of weights = 1) -> bv@Wp folds into the
    host-side residual.  The aug reduces to the shift row alone, published
    per stats checkpoint as an fp8 DRAM-bounced row into xsb8[47, h1].
  - The softmax denominator comes from a constant ones-COLUMN of the vaug
    tiles (one strided 72-element memset), not a data row.
  - exp strips split between ACT (true Exp, 1038ns) and DVE (Schraudolph
    fast-exp whose int8 result bits ARE the e4m3 encoding, 1192ns) by
    EXP_PATTERN strings balanced so both engines run ~equally loaded.
    (DMA cannot touch PSUM on trn2, so these are the only two engines that
    can read the score strips; everything else - scaling, squares, stats
    rows, residual add - is pushed to Pool/PE/queues.)
  - Everything else (stats via masked matmuls + Quake rsqrt, the q-side
    bf16 path with folded q/k projections, rowsum/postlude choreography)
    is inherited from the tuned v1.
"""

import numpy as np
import ml_dtypes

B, H, W, C = 2, 96, 96, 96
GROUPS = 32
EPS = 1e-5
HW = H * W                 # 9216
NCORES = 8
QCH = HW // 4              # 2304 query rows per core
GSPAN = HW // GROUPS       # 288 rows per group
QGROUPS = QCH // GSPAN     # 8 groups per query chunk
SCALE = float(C) ** -0.5
CA = C + 2                 # aug channels: 96=shift row, 97=zero pad
PAIRP = 49                 # pair partitions: 98 = 49 * 2
VA = C + 1                 # vaug cols: 96 = v, col 96 = ones (denominator)
VPAD = 112                 # vaug tile stride (16-aligned for DoubleRow pairs)

LOG2E = 1.4426950408889634
A_DVE = 8.0 * LOG2E * SCALE     # fast-exp: bits = floor(s*A + B) as e4m3
B_DVE = 56.0 - 1.16             # 8*7 bias, -1.16 tuned for min spread
NTILES = HW // 128         # 72 key tiles
NPAIRS = NTILES // 2       # 36 DoubleRow key pairs
CHK = 1152                 # 4 whole groups; preludes pipeline at this grain


def _pat(n, fA, pre=""):
    """Pattern string of length n: prefix then A/D alternating with
    A-fraction fA (never >2 in a row by construction for fA in [1/3,2/3])."""
    res = []
    accA = 0.0
    for _ in range(n - len(pre)):
        accA += fA
        if accA >= 1.0:
            res.append("A")
            accA -= 1.0
        else:
            res.append("D")
    return pre + "".join(res)


# per-m-block exp-engine patterns. Block bridges pre-emit the next block's
# first strips ACT-only so the previous postlude (DVE) isn't queued behind
# DVE exps.
EXP_PATTERN_MB = [
    _pat(72, 39 / 72.0),
    _pat(72, 32 / 66.0, pre="AAAAAA"),
    _pat(18, 8 / 16.0, pre="AA"),
]

_compiled = {}


def _build_bass():
    import concourse.bass as bass
    import concourse.mybir as mybir
    import concourse.tile as tile

    # --- workaround: TRN2 allows one embedded sem-wait per instruction, but
    # TileContext piles every outstanding DMA-queue wait onto one tail drain.
    import bass_rust

    def _split_drain_and_barrier(self, tick_clock, wait_clock):
        nc = self.nc
        drain_inst = nc.sync.drain()
        wait_clock.add_sem_waits(
            drain_inst.ins, bass_rust.ScopedClock({None: tick_clock.global_clock})
        )
        si = drain_inst.ins.sync_info
        waits = list(si.on_wait) if si is not None and si.on_wait else []
        if len(waits) > 1:
            si.on_wait = waits[:1]
            for w in waits[1:]:
                extra = nc.sync.drain()
                esi = extra.ins.sync_info
                if esi is None:
                    extra.ins.sync_info = bass_rust.SyncInfo(on_wait=[w], on_update=[])
                else:
                    esi.on_wait = [w]
        nc.all_engine_barrier()
        assert self.sems is not None
        popped = nc._tile_sem_poison_stack.pop()
        assert popped is self._sem_poison
        nc.clear_and_free_semaphores(list(self.sems.allocated().values()))
        nc.all_engine_barrier()

    tile.TileContext._drain_and_barrier = _split_drain_and_barrier

    def _split_multiwaits(nc):
        """TRN2 ISA allows one embedded sem-wait per instruction; Tile's
        sem-assignment sometimes attaches several. Hoist extras onto
        engine-NOPs spliced immediately before the instruction."""
        n_split = 0
        for f in nc.m.functions:
            for bb in f.blocks:
                out = []
                changed = False
                for inst in bb.instructions:
                    si = getattr(inst, "sync_info", None)
                    if si is not None and si.on_wait and len(si.on_wait) > 1:
                        waits = list(si.on_wait)
                        for w in waits[:-1]:
                            n_split += 1
                            nop = bass_rust.InstNoOp(
                                name=f"WSPLIT-{n_split}", ins=[], outs=[]
                            )
                            nop.engine = inst.engine
                            nop.sync_info = bass_rust.SyncInfo(
                                on_wait=[w], on_update=[]
                            )
                            nc.register_instruction(nop)
                            out.append(nop)
                        si.on_wait = waits[-1:]
                        changed = True
                    out.append(inst)
                if changed:
                    bb.instructions = out
        return n_split

    f32 = mybir.dt.float32
    bf16 = mybir.dt.bfloat16
    f8e4 = mybir.dt.float8e4
    i8 = mybir.dt.int8
    i32 = mybir.dt.int32
    AF = mybir.ActivationFunctionType
    ALU = mybir.AluOpType
    AX = mybir.AxisListType
    DR = mybir.MatmulPerfMode.DoubleRow

    nc = bass.Bass()

    xbT16 = nc.dram_tensor("xbT16", [C, HW], bf16, kind="ExternalInput")
    x8pD = nc.dram_tensor("x8pD", [PAIRP, 2 * HW], f8e4, kind="ExternalInput")
    xqT16 = nc.dram_tensor("xqT16", [C, QCH], bf16, kind="ExternalInput")
    xqT = nc.dram_tensor("xqT", [C, QCH], f32, kind="ExternalInput")
    gRow = nc.dram_tensor("gRow", [GROUPS, GSPAN], f32, kind="ExternalInput")
    bRow = nc.dram_tensor("bRow", [GROUPS, GSPAN], f32, kind="ExternalInput")
    gRowQ = nc.dram_tensor("gRowQ", [QGROUPS, GSPAN], f32, kind="ExternalInput")
    bRowQ = nc.dram_tensor("bRowQ", [QGROUPS, GSPAN], f32, kind="ExternalInput")
    WfoldD = nc.dram_tensor("WfoldD", [CA, CA], bf16, kind="ExternalInput")
    WvAugPD = nc.dram_tensor("WvAugPD", [PAIRP, 2 * C], f8e4,
                             kind="ExternalInput")
    Wp = nc.dram_tensor("Wp", [C, C], bf16, kind="ExternalInput")
    masksBD = nc.dram_tensor("masksBD", [C, GROUPS * GROUPS], bf16,
                             kind="ExternalInput")
    masksLD = nc.dram_tensor("masksLD", [C, QGROUPS * QGROUPS], bf16,
                             kind="ExternalInput")
    outT = nc.dram_tensor("outT", [C, QCH], f32, kind="ExternalOutput")
    # internal DRAM bounces (one tensor per checkpoint: whole-tensor deps)
    scRowD = [nc.dram_tensor(f"scRowD{j}", [HW], f8e4) for j in range(5)]
    shRowD = [nc.dram_tensor(f"shRowD{j}", [HW], f8e4) for j in range(5)]
    scRowQD = nc.dram_tensor("scRowQD", [QCH], bf16)
    rrD = [nc.dram_tensor(f"rrD{j}", [1024], bf16) for j in range(3)]

    with tile.TileContext(nc) as tc:
        import contextlib

        with contextlib.ExitStack() as ctx:
            consts = ctx.enter_context(tc.tile_pool(name="consts", bufs=1))
            big = ctx.enter_context(tc.tile_pool(name="big", bufs=1))
            # ALL psum comes from these two pools; disjoint regions: strips +
            # prelude/postlude share one 3-deep tag (6 banks), oT (2 banks).
            sps = ctx.enter_context(tc.tile_pool(name="sps", bufs=3, space="PSUM"))
            ops = ctx.enter_context(tc.tile_pool(name="ot_ps", bufs=1, space="PSUM"))
            sqp = ctx.enter_context(tc.tile_pool(name="sq_sb", bufs=2))
            stb = ctx.enter_context(tc.tile_pool(name="stat_sb", bufs=2))
            esb = ctx.enter_context(tc.tile_pool(name="exp_sb", bufs=6))
            osb = ctx.enter_context(tc.tile_pool(name="post_sb", bufs=2))

            # ---- big SBUF tensors (declared early; loads get top priority) --
            xb16 = big.tile([C, HW], bf16)       # raw bf16 x (batch stats)
            xq16 = big.tile([C, QCH], bf16)
            xqT_s = big.tile([C, QCH], f32)      # fp32 x kept only for residual
            x8p = big.tile([PAIRP, 2 * HW], f8e4)   # raw fp8 x, channel pairs
            xsb8 = big.tile([PAIRP, 2 * HW], f8e4)  # normalized fp8 pairs
            xsq = big.tile([CA, QCH], bf16)      # normalized local query chunk
            xsq_ones = xsq[C : C + 2, :]
            ones96b = consts.tile([VA, C], bf16)

            wf_t = consts.tile([CA, CA], bf16)
            wvap = consts.tile([PAIRP, 2 * C], f8e4)
            wp_t = consts.tile([C, C], bf16)
            masksB = consts.tile([C, GROUPS * GROUPS], bf16)
            masksL = consts.tile([C, QGROUPS * QGROUPS], bf16)

            gt_l = consts.tile([QGROUPS, GSPAN], f32, name="grow_L")
            bt_l = consts.tile([QGROUPS, GSPAN], f32, name="brow_L")
            gt_b = consts.tile([GROUPS, GSPAN], f32, name="grow_B")
            bt_b = consts.tile([GROUPS, GSPAN], f32, name="brow_B")

            # transfers serialize per issuing queue; order each queue by when
            # its payloads unblock compute. x8p chunk i covers key tiles
            # 9i..9i+8 in BOTH halves via a 3d AP.
            def chunk(dst, src, i):
                sl = slice(i * CHK, (i + 1) * CHK)
                return dst[:, sl], src[:, sl]

            def x8chunk(i):
                dst = x8p[:, :].rearrange("p (i n) -> p i n", i=2)[
                    :, :, i * CHK : (i + 1) * CHK
                ]
                src = bass.AP(
                    tensor=x8pD, offset=i * CHK,
                    ap=[[2 * HW, PAIRP], [HW, 2], [1, CHK]],
                )
                return dst, src

            # Only the ramp-critical minimum is loaded up front; bulk loads
            # are emitted inside the prelude loops so the per-queue FIFOs
            # keep the stats->scale-row->broadcast chains near their heads.
            for eng, dst, src in [
                (nc.sync, *chunk(xq16, xqT16, 0)),
                (nc.sync, masksL, masksLD[:, :]),
                (nc.sync, masksB, masksBD[:, :]),
                (nc.sync, *chunk(xq16, xqT16, 1)),
                (nc.sync, wf_t, WfoldD[:, :]),
                (nc.sync, gt_l, gRowQ[:, :]),
                (nc.sync, bt_l, bRowQ[:, :]),
                (nc.sync, gt_b, gRow[:, :]),
                (nc.sync, bt_b, bRow[:, :]),
                (nc.scalar, *chunk(xb16, xbT16, 0)),
                (nc.scalar, *chunk(xb16, xbT16, 1)),
                (None, None, None),  # ones-row memset slot (see below)
                (nc.scalar, *chunk(xb16, xbT16, 2)),
                (nc.scalar, *chunk(xb16, xbT16, 3)),
                (nc.sync, *x8chunk(0)),
            ]:
                if eng is None:
                    # xsq aug rows: ones; shift-row DMA overwrites row 96.
                    # vaug denominator column: constant ones at col 96 of
                    # every key tile (strided, 72 elements total).
                    nc.gpsimd.memset(xsq_ones, 1.0)
                    nc.gpsimd.memset(ones96b[C : C + 1, :], 1.0)
                    continue
                eng.dma_start(out=dst, in_=src)

            # dummy exp to trigger the ACT table load while DMAs run
            dumm = consts.tile([1, 8], f32)
            nc.vector.memset(dumm, 0.0)
            dumm2 = consts.tile([1, 8], f32)
            nc.scalar.activation(dumm2, dumm, AF.Exp)

            # ---- big SBUF tensors ----
            qT2p = big.tile([PAIRP, 2 * QCH], f8e4)  # fp8 folded q, pair halves
            vaug = big.tile([128, NTILES * VPAD], f8e4)
            oTr = big.tile([VA, QCH], bf16)      # attn@v evac; row 96 = rowsum
            rrB = big.tile([VA, QCH], bf16)      # 1/rowsum staged on part 96

            nc.gpsimd.memset(
                vaug[:, :].rearrange("p (t m) -> p t m", m=VPAD)[:, :, C : C + 1],
                1.0,
            )

            x8v = x8p[:, :].rearrange("p (i n) -> p i n", i=2)
            xs8v = xsb8[:, :].rearrange("p (i n) -> p i n", i=2)
            qT2v = qT2p[:, :].rearrange("p (i n) -> p i n", i=2)
            wvav = wvap[:, :].rearrange("p (i n) -> p i n", i=2)

            CNT = 1.0 / (GSPAN * C)
            SUBW = GSPAN // 4

            stats_ts = {}

            def stats_chunk(x16, key, i, first, last):
                """Colsums of groups 4i..4i+3 -> rows 4i+j of the side's
                persistent stats psum tile (lane-aligned)."""
                masks, ng = (masksL, QGROUPS) if key == "L" else (masksB, GROUPS)
                if first:
                    stats_ts[key] = ops.tile([VA, 1024], f32, tag="oT",
                                             name="ts")
                ts_ = stats_ts[key]
                chunk = x16[:, i * CHK : (i + 1) * CHK]
                sq = sqp.tile([C, CHK], bf16, tag="sq", name="sq")
                # bf16 squares hit the DVE 2x_1p mode (660ns vs 960 Pool);
                # early batch chunks ride DVE's prelude idle, late ones go
                # to Pool so DVE exps aren't delayed.
                sq_eng = nc.vector if (key == "L" or i < 2) else nc.gpsimd
                sq_eng.tensor_mul(sq, chunk, chunk)
                ps_s = ts_[0:ng, 0:SUBW]
                ps_q = ts_[0:ng, 512 : 512 + SUBW]
                for j in range(4):
                    g = 4 * i + j
                    mk = masks[:, g * ng : (g + 1) * ng]
                    for u in range(4):
                        sspan = slice(
                            j * GSPAN + u * SUBW, j * GSPAN + (u + 1) * SUBW
                        )
                        st_ = first and (j == 0 and u == 0)
                        sp_ = last and (j == 3 and u == 3)
                        nc.tensor.matmul(
                            ps_s, mk, chunk[:, sspan], start=st_, stop=sp_,
                            skip_group_check=True,
                        )
                        nc.tensor.matmul(
                            ps_q, mk, sq[:, sspan], start=st_, stop=sp_,
                            skip_group_check=True,
                        )

            fin_prev = {"L": 0, "B": 0}

            def stats_core(key, k, ve=None):
                """Reduce the running sums for groups 0:k and compute
                mean/var/rstd (Quake seed + 1 Newton step). ve picks the
                scalar-chain engine: DVE for ramp-critical checkpoints,
                Pool for late ones (DVE is exp-saturated by then)."""
                ve = ve or nc.vector
                acc = stb.tile([GROUPS, 2], f32, tag="red", name="red")[:k]
                both = stats_ts[key][0:k, :].rearrange(
                    "p (a s) -> p a s", a=2
                )[:, :, 0:SUBW]
                nc.vector.tensor_reduce(acc, both, axis=AX.X, op=ALU.add)
                st = stb.tile([GROUPS, 12], f32, tag="st", name="st")[:k]
                mean, ex2 = st[:, 0:1], st[:, 1:2]
                msq, var = st[:, 2:3], st[:, 3:4]
                veps, ti = st[:, 4:5], st[:, 5:6]
                ya, yb = st[:, 6:7], st[:, 7:8]
                rstd = st[:, 8:9]
                ve.tensor_scalar_mul(mean, in0=acc[:, 0:1], scalar1=CNT)
                ve.tensor_scalar_mul(ex2, in0=acc[:, 1:2], scalar1=CNT)
                ve.tensor_mul(msq, mean, mean)
                ve.tensor_sub(var, ex2, msq)
                ve.tensor_scalar_add(veps, in0=var, scalar1=EPS)
                ve.tensor_scalar(
                    out=ti.bitcast(i32), in0=veps.bitcast(i32),
                    scalar1=1, scalar2=-1, op0=ALU.arith_shift_right,
                    op1=ALU.bitwise_xor,
                )
                ve.tensor_scalar_add(
                    rstd.bitcast(i32), in0=ti.bitcast(i32), scalar1=0x5F3759E0
                )
                ve.tensor_mul(ya, rstd, rstd)
                ve.tensor_mul(yb, ya, veps)
                ve.tensor_scalar(
                    out=yb, in0=yb, scalar1=-0.5, scalar2=1.5,
                    op0=ALU.mult, op1=ALU.add,
                )
                ve.tensor_mul(rstd, rstd, yb)
                return st

            def finish_L(k):
                """Query-side checkpoint: scale/shift rows for the bf16 xsq
                path, delta groups kp:k only (the q-side groups split 4/4
                across the two chunks, so chunk 0's rows publish early)."""
                st = stats_core("L", k)
                mean, rstd = st[:, 0:1], st[:, 8:9]
                # compute ops must start at partition 0 (hw 0/32/64/96
                # rule), so recompute the [0:k] prefix and re-publish it -
                # values are identical, the redundant bytes are cheap.
                sc16 = stb.tile([GROUPS, GSPAN], bf16, tag="sc16",
                                name="sc16")[:k]
                nc.vector.tensor_scalar_mul(sc16, in0=gt_l[:k], scalar1=rstd)
                ms32 = stb.tile([GROUPS, GSPAN], f32, tag="ms",
                                name="ms32")[:k]
                nc.vector.tensor_scalar_mul(ms32, in0=gt_l[:k], scalar1=rstd)
                nc.vector.tensor_scalar_mul(ms32, in0=ms32, scalar1=mean)
                sh16 = stb.tile([GROUPS, GSPAN], bf16, tag="sh16",
                                name="sh16")[:k]
                nc.vector.tensor_sub(sh16, bt_l[:k], ms32)
                nc.sync.dma_start(
                    out=scRowQD[0 : k * GSPAN].rearrange(
                        "(g s) -> g s", s=GSPAN
                    ),
                    in_=sc16,
                )
                nc.gpsimd.dma_start(
                    out=xsq[C : C + 1, 0 : k * GSPAN].rearrange(
                        "p (g s) -> p g s", g=k
                    ),
                    in_=sh16.rearrange("g (a s) -> g a s", a=1),
                )

            def finish_B(j, k, ve=None):
                """Batch-side checkpoint: publish the fp8 scale row (pulled
                back per chunk as a pair broadcast) and the fp8 shift row
                (bounced into xsb8[48, h1]) for groups kp:k."""
                ve = ve or nc.vector
                st = stats_core("B", k, ve)
                kp = fin_prev["B"]
                fin_prev["B"] = k
                mean, rstd = st[:, 0:1], st[:, 8:9]
                sk = slice(kp, k)
                d = k - kp
                # DVE computes the f32 rows; SWDGE (gpsimd) DMAs cast
                # f32 -> fp8 while publishing, skipping a staging hop
                sc32 = stb.tile([GROUPS, GSPAN], f32, tag="sc", name="sc32")
                ve.tensor_scalar_mul(
                    sc32[:k], in0=gt_b[:k], scalar1=rstd[:k]
                )
                ms32 = stb.tile([GROUPS, GSPAN], f32, tag="ms", name="ms32")
                ve.tensor_scalar_mul(
                    ms32[:k], in0=sc32[:k], scalar1=mean[:k]
                )
                sh32 = stb.tile([GROUPS, GSPAN], f32, tag="sh", name="sh32")
                ve.tensor_sub(sh32[:k], bt_b[:k], ms32[:k])
                nc.gpsimd.dma_start(
                    out=bass.AP(tensor=scRowD[j], offset=kp * GSPAN,
                                ap=[[GSPAN, d], [1, GSPAN]]),
                    in_=sc32[sk],
                )
                nc.gpsimd.dma_start(
                    out=bass.AP(tensor=shRowD[j], offset=kp * GSPAN,
                                ap=[[GSPAN, d], [1, GSPAN]]),
                    in_=sh32[sk],
                )

            def shrow_bounce(j, kp, k):
                """Pull the published shift row into the aug channel (pair
                slot 48/h1). The scale multiplies never touch that row, so
                this bounce runs concurrently with them."""
                nc.sync.dma_start(
                    out=xsb8[PAIRP - 1 : PAIRP,
                             HW + kp * GSPAN : HW + k * GSPAN],
                    in_=bass.AP(tensor=shRowD[j], offset=kp * GSPAN,
                                ap=[[HW, 1], [1, (k - kp) * GSPAN]]),
                )

            def scaled_chunk_B(i, j, eng=None):
                """xsb8 = x8p * scale for chunk i (both halves in one op);
                scale comes back as a 49-partition x 2-half broadcast of
                the published fp8 row. Default Pool; the first chunks ride
                DVE's prelude idle to shorten the Pool serial chain."""
                scb = sqp.tile([PAIRP, 2 * CHK], f8e4, tag="scb", name="scb")
                bcast_src = bass.AP(
                    tensor=scRowD[j], offset=i * CHK,
                    ap=[[0, PAIRP], [0, 2], [1, CHK]],
                )
                nc.sync.dma_start(
                    out=scb[:, :].rearrange("p (i n) -> p i n", i=2),
                    in_=bcast_src,
                )
                e_ = eng or nc.gpsimd
                e_.tensor_mul(
                    xs8v[:, 0, i * CHK : (i + 1) * CHK],
                    x8v[:, 0, i * CHK : (i + 1) * CHK],
                    scb[:, 0:CHK],
                )
                e_.tensor_mul(
                    xs8v[0 : PAIRP - 1, 1, i * CHK : (i + 1) * CHK],
                    x8v[0 : PAIRP - 1, 1, i * CHK : (i + 1) * CHK],
                    scb[0 : PAIRP - 1, CHK : 2 * CHK],
                )

            def scaled_chunk_L(i):
                """xsq = xq16 * scaleQ via a partition-broadcast DMA."""
                sl = slice(i * CHK, (i + 1) * CHK)
                scb = sqp.tile([C, CHK], bf16, tag="scbL", name="scbL")
                bcast_src = bass.AP(
                    tensor=scRowQD, offset=i * CHK, ap=[[0, C], [1, CHK]]
                )
                nc.sync.dma_start(out=scb, in_=bcast_src)
                nc.gpsimd.tensor_mul(xsq[0:C, sl], xq16[:, sl], scb)

            def emit_vaug(i):
                """v-projection for chunk i's 9 key tiles: fp8 DoubleRow
                matmuls from the normalized pair tensor; plain fp8 evacs
                write cols 0:96 (col 96 is the constant ones column)."""
                t0 = i * 9
                for base, cnt in [(0, 5), (5, 4)]:
                    tv = sps.tile([128, 512], f32, tag="sp", name="tv")
                    for jj in range(cnt):
                        t = t0 + base + jj
                        nc.tensor.matmul(
                            tv[:, jj * C : (jj + 1) * C],
                            xs8v[:, :, t * 128 : (t + 1) * 128],
                            wvav,
                            start=True, stop=True, perf_mode=DR,
                        )
                    nc.scalar.activation(
                        vaug[
                            :, (t0 + base) * VPAD : (t0 + base + cnt) * VPAD
                        ].rearrange("p (t m) -> p t m", t=cnt)[:, :, 0:C],
                        tv[:, 0 : cnt * C].rearrange(
                            "p (t m) -> p t m", t=cnt
                        ),
                        AF.Copy,
                    )

            # ---- local (query) prelude ----
            # qT2p = fp8 pair-layout (Wfold^T xsq): two half-column matmuls
            # per span into one [49, 1024] psum, one ACT evac each. The
            # q-side groups split 4/4 across the chunks, so each chunk's
            # scale rows publish as soon as its own stats land.
            for i in range(2):
                stats_chunk(xq16, "L", i, first=(i == 0), last=(i == 1))
                finish_L(4 * (i + 1))
                scaled_chunk_L(i)
                for off in (0, 512, 1024):
                    lo = i * CHK + off
                    w = min(512, (i + 1) * CHK - lo)
                    tq = sps.tile([128, 1024], f32, tag="sp", name="tq")
                    for half in range(2):
                        nc.tensor.matmul(
                            tq[0:PAIRP, half * 512 : half * 512 + w],
                            wf_t[:, half * PAIRP : (half + 1) * PAIRP],
                            xsq[:, lo : lo + w],
                            start=True, stop=True,
                        )
                    nc.scalar.activation(
                        qT2v[:, :, lo : lo + w],
                        tq[0:PAIRP, :].rearrange("p (i n) -> p i n", i=2)[
                            :, :, 0:w
                        ],
                        AF.Copy,
                    )

            # ---- batch prelude: stats -> scale rows -> xsb8 per chunk.
            # Four checkpoints so the tail chunks' scaling isn't gated on
            # the full-batch stats; the x8p tail loads are emitted AFTER
            # each checkpoint's critical scale-row broadcasts (queue FIFO).
            for i in range(8):
                stats_chunk(xb16, "B", i, first=(i == 0), last=(i == 7))
                if i == 0:
                    finish_B(0, 4)
                    scaled_chunk_B(0, 0, nc.vector)
                    shrow_bounce(0, 0, 4)
                    for eng, dst, src in [
                        (nc.sync, *x8chunk(1)),
                        (nc.scalar, *chunk(xb16, xbT16, 4)),
                        (nc.scalar, *chunk(xb16, xbT16, 5)),
                    ]:
                        eng.dma_start(out=dst, in_=src)
                elif i == 1:
                    finish_B(1, 8)
                    scaled_chunk_B(1, 1)
                    shrow_bounce(1, 4, 8)
                    for eng, dst, src in [
                        (nc.sync, wvap, WvAugPD[:, :]),
                        (nc.sync, *x8chunk(2)),
                        (nc.sync, *x8chunk(3)),
                        (nc.sync, *chunk(xb16, xbT16, 6)),
                        (nc.sync, *chunk(xb16, xbT16, 7)),
                    ]:
                        eng.dma_start(out=dst, in_=src)
                    for jj in (0, 1):
                        emit_vaug(jj)
                elif i == 3:
                    finish_B(2, 16)
                    scaled_chunk_B(2, 2)
                    scaled_chunk_B(3, 2)
                    shrow_bounce(2, 8, 16)
                    for eng, dst, src in [
                        (nc.sync, *x8chunk(4)),
                        (nc.sync, *x8chunk(5)),
                    ]:
                        eng.dma_start(out=dst, in_=src)
                elif i == 5:
                    finish_B(3, 24)
                    scaled_chunk_B(4, 3)
                    scaled_chunk_B(5, 3)
                    shrow_bounce(3, 16, 24)
                    for eng, dst, src in [
                        (nc.sync, *x8chunk(6)),
                        (nc.sync, *x8chunk(7)),
                    ]:
                        eng.dma_start(out=dst, in_=src)
                elif i == 7:
                    finish_B(4, GROUPS)
                    scaled_chunk_B(6, 4)
                    scaled_chunk_B(7, 4)
                    shrow_bounce(4, 24, GROUPS)

            # postlude-only loads, emitted here so they sit behind the
            # stats chain in the gpsimd queue
            for dst, src_ in [
                (xqT_s[:, 0:CHK], xqT[:, 0:CHK]),
                (xqT_s[:, CHK : 2 * CHK], xqT[:, CHK : 2 * CHK]),
                (wp_t, Wp[:, :]),
            ]:
                nc.gpsimd.dma_start(out=dst, in_=src_)

            mb_count = [0]

            def mb_open(mw):
                pattern = EXP_PATTERN_MB[mb_count[0]]
                mb_count[0] += 1
                return {
                    "oT": ops.tile([VA, 1024], f32, tag="oT", name="oT"),
                    "pend": [], "next": 0,
                    "spb": 1024 // mw,
                    "halves": [(h, min(512, mw - h)) for h in range(0, mw, 512)],
                    "ex_half": [None],
                    "pattern": pattern, "ctr": [0],
                }

            def mb_emit(st, mo, mw, upto_tile):
                spb, halves = st["spb"], st["halves"]
                nst = NTILES // spb
                while st["next"] < nst and st["next"] * spb < upto_tile:
                    s = st["next"]
                    sp = sps.tile([128, 1024], f32, tag="sp", name="sp")
                    for j in range(spb):
                        t = s * spb + j
                        for h, hw_ in halves:
                            nc.tensor.matmul(
                                sp[:, j * mw + h : j * mw + h + hw_],
                                xs8v[:, :, t * 128 : (t + 1) * 128],
                                qT2v[:, :, mo + h : mo + h + hw_],
                                start=True, stop=True, perf_mode=DR,
                            )
                    c = st["ctr"][0]
                    st["ctr"][0] += 1
                    eng_c = st["pattern"][c]
                    if spb == 1:
                        # pair tile [128, 2048]: halves = strips 2P, 2P+1
                        if st["ex_half"][0] is None:
                            ex = esb.tile([128, 2048], f8e4, tag="ex", name="ex")
                            st["ex_half"][0] = ex
                            dst = ex[:, 0:1024]
                            pair_done = False
                        else:
                            ex = st["ex_half"][0]
                            dst = ex[:, 1024:2048]
                            st["ex_half"][0] = None
                            pair_done = True
                    else:
                        ex = esb.tile([128, 1024], f8e4, tag="ex", name="ex")
                        dst = ex[:, 0:1024]
                        pair_done = True
                    if eng_c == "A":
                        nc.scalar.activation(
                            dst, sp[:, : spb * mw], AF.Exp, scale=SCALE
                        )
                    else:
                        nc.vector.tensor_scalar(
                            out=dst.bitcast(i8), in0=sp[:, : spb * mw],
                            scalar1=A_DVE, scalar2=B_DVE,
                            op0=ALU.mult, op1=ALU.add,
                        )
                    if pair_done:
                        st["pend"].append((s, ex))
                        if len(st["pend"]) > 3:
                            _mb_c(st, mo, mw)
                    st["next"] += 1

            def _mb_c(st, mo, mw):
                """DoubleRow attn@V for one ready pair group: contract 256
                keys per matmul (2 key-tiles in the free-dim pair axis)."""
                spb, halves = st["spb"], st["halves"]
                s_, ex_ = st["pend"].pop(0)
                if spb == 1:
                    P = s_ // 2
                    va = vaug[:, 2 * P * VPAD : (2 * P + 2) * VPAD].rearrange(
                        "p (i m) -> p i m", i=2
                    )[:, :, 0:VA]
                    exp_pair = ex_[:, :].rearrange("p (i n) -> p i n", i=2)
                    for h, hw_ in halves:
                        nc.tensor.matmul(
                            st["oT"][:, h : h + hw_],
                            va, exp_pair[:, :, h : h + hw_],
                            start=(P == 0), stop=(P == NPAIRS - 1),
                            perf_mode=DR,
                        )
                else:
                    for j in range(2):
                        P = 2 * s_ + j
                        va = vaug[:, 2 * P * VPAD : (2 * P + 2) * VPAD].rearrange(
                            "p (i m) -> p i m", i=2
                        )[:, :, 0:VA]
                        exp_pair = ex_[:, j * 2 * mw : (j + 1) * 2 * mw].rearrange(
                            "p (i n) -> p i n", i=2
                        )
                        nc.tensor.matmul(
                            st["oT"][:, 0:mw],
                            va, exp_pair,
                            start=(P == 0), stop=(P == NPAIRS - 1),
                            perf_mode=DR,
                        )

            mb_idx = [0]

            def mb_finish_head(st, mo, mw):
                """Drain the pair backlog and start the postlude chain
                (evac -> recip -> 1/r publish)."""
                while st["pend"]:
                    _mb_c(st, mo, mw)
                oT = st["oT"]
                rd = rrD[min(mb_idx[0], 2)]
                mb_idx[0] += 1
                final = mb_idx[0] == 3
                st["fin"] = (rd, final)
                nc.vector.tensor_copy(oTr[:, mo : mo + mw], oT[0:VA, :mw])
                with nc.allow_low_precision(reason="softmax denom; bf16 ok"):
                    nc.vector.reciprocal(
                        rrB[C : C + 1, mo : mo + mw],
                        oTr[C : C + 1, mo : mo + mw],
                    )
                if not final:
                    nc.sync.dma_start(
                        out=rd[0:mw].rearrange("(a n) -> a n", a=1),
                        in_=rrB[C : C + 1, mo : mo + mw],
                    )

            def mb_finish_body(st, mo, mw):
                rd, final = st["fin"]
                po = mo
                while po < mo + mw:
                    pw = min(512, mo + mw - po)
                    tc_ = sps.tile([128, 512], f32, tag="sp", name="tpost")
                    pp = tc_[0:C, 0:pw]
                    nc.tensor.matmul(
                        pp, wp_t, oTr[0:C, po : po + pw], start=True, stop=True
                    )
                    sc = osb.tile([C, 512], f32, tag="sc", name="sc")
                    if final:
                        tc2 = sps.tile([128, 512], f32, tag="sp", name="tpo2")
                        pr = tc2[0:C, 0:pw]
                        nc.tensor.matmul(
                            pr, ones96b[C : C + 1, :],
                            rrB[C : C + 1, po : po + pw],
                            start=True, stop=True, tile_position=(96, 0),
                        )
                        prs = osb.tile([C, 512], bf16, tag="rbc", name="prs")
                        nc.vector.tensor_copy(prs[:, :pw], pr)
                        nc.vector.tensor_mul(sc[:, :pw], pp, prs[:, :pw])
                    else:
                        rbc = osb.tile([C, 512], bf16, tag="rbc", name="rbc")
                        nc.sync.dma_start(
                            out=rbc[:, :pw],
                            in_=bass.AP(tensor=rd, offset=po - mo,
                                        ap=[[0, C], [1, pw]]),
                        )
                        nc.vector.tensor_mul(sc[:, :pw], pp, rbc[:, :pw])
                    ot = osb.tile([C, 512], f32, tag="ot", name="ot")
                    add_eng = nc.vector if final else nc.gpsimd
                    add_eng.tensor_add(
                        ot[:, :pw], sc[:, :pw], xqT_s[:, po : po + pw]
                    )
                    nc.sync.dma_start(out=outT[:, po : po + pw], in_=ot[:, :pw])
                    po += pw

            def mb_finish(st, mo, mw):
                mb_finish_head(st, mo, mw)
                mb_finish_body(st, mo, mw)

            st0 = mb_open(1024)
            for i in range(8):
                if i >= 2:
                    emit_vaug(i)
                mb_emit(st0, 0, 1024, 9 * (i + 1))
            # bridge m-block boundaries: pre-emit the next block's first
            # strips before draining this block's tail so ACT/DVE never
            # idle across the transition.
            st1 = mb_open(1024)
            mb_emit(st1, 1024, 1024, 4)
            mb_finish_head(st0, 0, 1024)
            mb_emit(st1, 1024, 1024, 28)
            mb_finish_body(st0, 0, 1024)
            mb_emit(st1, 1024, 1024, NTILES)
            st2 = mb_open(256)
            mb_emit(st2, 2048, 256, 8)
            mb_finish_head(st1, 1024, 1024)
            mb_emit(st2, 2048, 256, NTILES)
            mb_finish_body(st1, 1024, 1024)
            mb_finish(st2, 2048, 256)

    _split_multiwaits(nc)
    return nc


def _prep_inputs(x, gamma, beta, Wq, bq, Wk, bk, Wv, bv, Wp, bp):
    bf16 = ml_dtypes.bfloat16
    f8 = ml_dtypes.float8_e4m3fn if hasattr(ml_dtypes, "float8_e4m3fn") \
        else ml_dtypes.float8_e4m3
    f32 = np.float32

    x2 = np.ascontiguousarray(x.reshape(B, HW, C))
    gam = np.repeat(np.asarray(gamma, np.float64), W)      # [HW] per position
    bet = np.repeat(np.asarray(beta, np.float64), W)
    gRow = gam.reshape(GROUPS, GSPAN).astype(f32)
    bRow = bet.reshape(GROUPS, GSPAN).astype(f32)

    # WvAug: aug channels (96 Wv rows + shift-coeff row + zero pad), v cols
    # only - bv folds into the residual, the denominator is a vaug ones col.
    WvAug = np.zeros((CA, C), f32)
    WvAug[:C, :] = Wv
    WvAug[C, :] = Wv.sum(axis=0)       # u_v: shift-row coefficient
    WvAugP = np.zeros((PAIRP, 2 * C), f32)
    WvAugP[:, 0:C] = WvAug[0:PAIRP]
    WvAugP[0 : C - PAIRP, C : 2 * C] = WvAug[PAIRP:C]
    WvAugP[PAIRP - 1, C : 2 * C] = WvAug[C]

    def aug(Wm, bias):
        a = np.empty((CA, C), f32)
        a[:C] = Wm
        a[C] = Wm.sum(axis=0)
        a[C + 1] = bias
        return a

    WqAug = aug(np.asarray(Wq, f32), bq)
    WkAugT = np.ascontiguousarray(aug(np.asarray(Wk, f32), bk).T)
    Wfold = (WqAug.astype(np.float64) @ WkAugT.astype(np.float64)).astype(f32)
    # k-side slot permutation: pair slot (47,h1) = zero pad, (48,h1) = the
    # shift channel (so scale multiplies never touch the shift row); the
    # k-side bias channel is softmax-invariant and dropped entirely.
    Wf2 = np.zeros_like(Wfold)
    Wf2[:, 0:C] = Wfold[:, 0:C]
    Wf2[:, C + 1] = Wfold[:, C]
    Wfold = Wf2

    masksB = np.zeros((C, GROUPS * GROUPS), f32)
    for g in range(GROUPS):
        masksB[:, g * GROUPS + g] = 1.0
    masksL = np.zeros((C, QGROUPS * QGROUPS), f32)
    for g in range(QGROUPS):
        masksL[:, g * QGROUPS + g] = 1.0

    # residual with the output-projection bias AND bv@Wp pre-added (bv is
    # attention-invariant: softmax weights sum to 1)
    radd = (np.asarray(bp, np.float64)
            + np.asarray(bv, np.float64) @ np.asarray(Wp, np.float64))

    in_maps = []
    for core in range(NCORES):
        b, qc = divmod(core, 4)
        xbT = np.ascontiguousarray(x2[b].T)
        xqT = np.ascontiguousarray(xbT[:, qc * QCH : (qc + 1) * QCH])
        x8 = xbT.astype(f8)
        x8pD = np.zeros((PAIRP, 2 * HW), f8)
        x8pD[:, 0:HW] = x8[0:PAIRP]
        x8pD[0 : C - PAIRP, HW : 2 * HW] = x8[PAIRP:C]
        in_maps.append({
            "xbT16": xbT.astype(bf16),
            "x8pD": x8pD,
            "xqT16": xqT.astype(bf16),
            "xqT": (xqT.astype(np.float64)
                    + radd.reshape(C, 1)).astype(f32),
            "gRow": gRow, "bRow": bRow,
            "gRowQ": np.ascontiguousarray(
                gam.reshape(4, QGROUPS, GSPAN)[qc].astype(f32)),
            "bRowQ": np.ascontiguousarray(
                bet.reshape(4, QGROUPS, GSPAN)[qc].astype(f32)),
            "WfoldD": Wfold.astype(bf16),
            "WvAugPD": WvAugP.astype(f8),
            "Wp": np.asarray(Wp, f32).astype(bf16),
            "masksBD": masksB.astype(bf16),
            "masksLD": masksL.astype(bf16),
        })
    return in_maps


def _get_sharded_fn():
    """Build the 8-core shard_map callable once so repeated calls reuse the
    compiled NEFF executable."""
    if "fn" in _compiled:
        return _compiled["fn"]

    import jax
    from jax.sharding import Mesh, PartitionSpec
    from jax.experimental.shard_map import shard_map
    import concourse.mybir as mybir
    from concourse.bass2jax import (
        _bass_exec_p, install_neuronx_cc_hook, partition_id_tensor
    )

    if "nc" not in _compiled:
        _compiled["nc"] = _build_bass()
    nc = _compiled["nc"]
    install_neuronx_cc_hook()

    pname = nc.partition_id_tensor.name if nc.partition_id_tensor else None
    in_names, out_names, out_avals = [], [], []
    for alloc in nc.m.functions[0].allocations:
        if not isinstance(alloc, mybir.MemoryLocationSet):
            continue
        name = alloc.memorylocations[0].name
        if alloc.kind == "ExternalInput":
            if name != pname:
                in_names.append(name)
        elif alloc.kind == "ExternalOutput":
            out_names.append(name)
            out_avals.append(
                jax.core.ShapedArray(
                    tuple(alloc.tensor_shape), mybir.dt.np(alloc.dtype)
                )
            )
    n_params = len(in_names)
    all_names = in_names + out_names
    if pname is not None:
        all_names = all_names + [pname]

    def _body(*args):
        operands = list(args)
        if pname is not None:
            operands.append(partition_id_tensor())
        outs = _bass_exec_p.bind(
            *operands,
            out_avals=tuple(out_avals),
            in_names=tuple(all_names),
            out_names=tuple(out_names),
            lowering_input_output_aliases=(),
            sim_require_finite=True,
            sim_require_nnan=True,
            nc=nc,
        )
        return tuple(outs)

    devices = jax.devices()[:NCORES]
    mesh = Mesh(np.asarray(devices), ("core",))
    sharded = jax.jit(
        shard_map(
            _body, mesh=mesh,
            in_specs=(PartitionSpec("core"),) * (n_params + len(out_names)),
            out_specs=(PartitionSpec("core"),) * len(out_names),
            check_rep=False,
        ),
        keep_unused=True,
    )

    from jax.sharding import NamedSharding

    shard = NamedSharding(mesh, PartitionSpec("core"))

    def put(in_maps):
        """Upload per-core inputs + zero outputs once; reuse across calls."""
        dev = [
            jax.device_put(
                np.concatenate(
                    [np.asarray(in_maps[c][nm]) for c in range(NCORES)], axis=0
                ),
                shard,
            )
            for nm in in_names
        ]
        dev += [
            jax.device_put(
                np.zeros((NCORES * a.shape[0], *a.shape[1:]), a.dtype), shard
            )
            for a in out_avals
        ]
        return dev

    def execute(dev_in):
        return sharded(*dev_in)

    def run(in_maps):
        out_arrs = execute(put(in_maps))
        return {
            nm: np.asarray(out_arrs[i]).reshape(NCORES, *out_avals[i].shape)
            for i, nm in enumerate(out_names)
        }

    _compiled["fn"] = (run, out_names, put, execute)
    _compiled["mkchain"] = (sharded, in_names, out_names, _body)
    return _compiled["fn"]


def kernel(x, gamma, beta, Wq, bq, Wk, bk, Wv, bv, Wp, bp):
    run = _get_sharded_fn()[0]
    in_maps = _prep_inputs(
        np.asarray(x, np.float32), gamma, beta, Wq, bq, Wk, bk, Wv, bv, Wp, bp
    )
    res = run(in_maps)["outT"]

    out = np.empty((B, HW, C), np.float32)
    for core in range(NCORES):
        b, qc = divmod(core, 4)
        out[b, qc * QCH : (qc + 1) * QCH, :] = res[core].T
    return out.reshape(B, H, W, C)


# revision 74
# speedup vs baseline: 1.0380x; 1.0380x over previous
"""Trainium2 Bass kernel for nn_AttnBlock (GroupNorm + dense spatial attention).

Reference math (B=2, H=W=C=96, GROUPS=32, fp32):
    hn = GroupNorm32 over dim1(H) of x[B,H,W,C]  (stats over (3,W,C) per group)
    q/k/v = hn @ W* + b*;  scores = (q @ k^T)/sqrt(C) over HW=9216 per batch
    out = x + softmax(scores) @ v @ Wp + bp

Sharding (8 cores): core = (b, qc), b = core//4, qc = core%4. Each core holds
the full batch-b tensors (for K/V) plus its 2304-query-row chunk.

Design (v3 - fp8 DoubleRow PE + exp-balanced ACT/DVE):
  - The k-side tensor ships as RAW fp8e4 x in a channel-pair layout
    x8p[49, 2*HW] (channel c = i*49+p), normalized on-device into xsb8 by
    Pool (otherwise idle) from fp8 scale-row broadcasts.  Every contraction
    over the 97 aug channels then runs as a DoubleRow fp8 matmul at 0.5
    cycles/col - half the PE time of bf16 - for the score matmuls, the
    v-projection, and (existing key-pair trick) attn@V.
  - The k-side BIAS term of the scores is a per-query additive constant,
    softmax-invariant -> dropped (no ones row; pair slot 48/h1 is zero).
    bv is attention-invariant (sum of weights = 1) -> bv@Wp folds into the
    host-side residual.  The aug reduces to the shift row alone, published
    per stats checkpoint as an fp8 DRAM-bounced row into xsb8[47, h1].
  - The softmax denominator comes from a constant ones-COLUMN of the vaug
    tiles (one strided 72-element memset), not a data row.
  - exp strips split between ACT (true Exp, 1038ns) and DVE (Schraudolph
    fast-exp whose int8 result bits ARE the e4m3 encoding, 1192ns) by
    EXP_PATTERN strings balanced so both engines run ~equally loaded.
    (DMA cannot touch PSUM on trn2, so these are the only two engines that
    can read the score strips; everything else - scaling, squares, stats
    rows, residual add - is pushed to Pool/PE/queues.)
  - Everything else (stats via masked matmuls + Quake rsqrt, the q-side
    bf16 path with folded q/k projections, rowsum/postlude choreography)
    is inherited from the tuned v1.
"""

import numpy as np
import ml_dtypes

B, H, W, C = 2, 96, 96, 96
GROUPS = 32
EPS = 1e-5
HW = H * W                 # 9216
NCORES = 8
QCH = HW // 4              # 2304 query rows per core
GSPAN = HW // GROUPS       # 288 rows per group
QGROUPS = QCH // GSPAN     # 8 groups per query chunk
SCALE = float(C) ** -0.5
CA = C + 2                 # aug channels: 96=shift row, 97=zero pad
PAIRP = 49                 # pair partitions: 98 = 49 * 2
VA = C + 1                 # vaug cols: 96 = v, col 96 = ones (denominator)
VPAD = 112                 # vaug tile stride (16-aligned for DoubleRow pairs)

LOG2E = 1.4426950408889634
A_DVE = 8.0 * LOG2E * SCALE     # fast-exp: bits = floor(s*A + B) as e4m3
B_DVE = 56.0 - 1.16             # 8*7 bias, -1.16 tuned for min spread
NTILES = HW // 128         # 72 key tiles
NPAIRS = NTILES // 2       # 36 DoubleRow key pairs
CHK = 1152                 # 4 whole groups; preludes pipeline at this grain


def _pat(n, fA, pre=""):
    """Pattern string of length n: prefix then A/D alternating with
    A-fraction fA (never >2 in a row by construction for fA in [1/3,2/3])."""
    res = []
    accA = 0.0
    for _ in range(n - len(pre)):
        accA += fA
        if accA >= 1.0:
            res.append("A")
            accA -= 1.0
        else:
            res.append("D")
    return pre + "".join(res)


# per-m-block exp-engine patterns. Block bridges pre-emit the next block's
# first strips ACT-only so the previous postlude (DVE) isn't queued behind
# DVE exps.
EXP_PATTERN_MB = [
    _pat(72, 39 / 72.0),
    _pat(72, 33 / 66.0, pre="AAAAAA"),
    _pat(18, 10 / 16.0, pre="AA"),
]

_compiled = {}


def _build_bass():
    import concourse.bass as bass
    import concourse.mybir as mybir
    import concourse.tile as tile

    # --- workaround: TRN2 allows one embedded sem-wait per instruction, but
    # TileContext piles every outstanding DMA-queue wait onto one tail drain.
    import bass_rust

    def _split_drain_and_barrier(self, tick_clock, wait_clock):
        nc = self.nc
        drain_inst = nc.sync.drain()
        wait_clock.add_sem_waits(
            drain_inst.ins, bass_rust.ScopedClock({None: tick_clock.global_clock})
        )
        si = drain_inst.ins.sync_info
        waits = list(si.on_wait) if si is not None and si.on_wait else []
        if len(waits) > 1:
            si.on_wait = waits[:1]
            for w in waits[1:]:
                extra = nc.sync.drain()
                esi = extra.ins.sync_info
                if esi is None:
                    extra.ins.sync_info = bass_rust.SyncInfo(on_wait=[w], on_update=[])
                else:
                    esi.on_wait = [w]
        nc.all_engine_barrier()
        assert self.sems is not None
        popped = nc._tile_sem_poison_stack.pop()
        assert popped is self._sem_poison
        nc.clear_and_free_semaphores(list(self.sems.allocated().values()))
        nc.all_engine_barrier()

    tile.TileContext._drain_and_barrier = _split_drain_and_barrier

    def _split_multiwaits(nc):
        """TRN2 ISA allows one embedded sem-wait per instruction; Tile's
        sem-assignment sometimes attaches several. Hoist extras onto
        engine-NOPs spliced immediately before the instruction."""
        n_split = 0
        for f in nc.m.functions:
            for bb in f.blocks:
                out = []
                changed = False
                for inst in bb.instructions:
                    si = getattr(inst, "sync_info", None)
                    if si is not None and si.on_wait and len(si.on_wait) > 1:
                        waits = list(si.on_wait)
                        for w in waits[:-1]:
                            n_split += 1
                            nop = bass_rust.InstNoOp(
                                name=f"WSPLIT-{n_split}", ins=[], outs=[]
                            )
                            nop.engine = inst.engine
                            nop.sync_info = bass_rust.SyncInfo(
                                on_wait=[w], on_update=[]
                            )
                            nc.register_instruction(nop)
                            out.append(nop)
                        si.on_wait = waits[-1:]
                        changed = True
                    out.append(inst)
                if changed:
                    bb.instructions = out
        return n_split

    f32 = mybir.dt.float32
    bf16 = mybir.dt.bfloat16
    f8e4 = mybir.dt.float8e4
    i8 = mybir.dt.int8
    i32 = mybir.dt.int32
    AF = mybir.ActivationFunctionType
    ALU = mybir.AluOpType
    AX = mybir.AxisListType
    DR = mybir.MatmulPerfMode.DoubleRow

    nc = bass.Bass()

    xbT16 = nc.dram_tensor("xbT16", [C, HW], bf16, kind="ExternalInput")
    x8pD = nc.dram_tensor("x8pD", [PAIRP, 2 * HW], f8e4, kind="ExternalInput")
    xqT16 = nc.dram_tensor("xqT16", [C, QCH], bf16, kind="ExternalInput")
    xqT = nc.dram_tensor("xqT", [C, QCH], f32, kind="ExternalInput")
    gRow = nc.dram_tensor("gRow", [GROUPS, GSPAN], f32, kind="ExternalInput")
    bRow = nc.dram_tensor("bRow", [GROUPS, GSPAN], f32, kind="ExternalInput")
    WfoldD = nc.dram_tensor("WfoldD", [CA, CA], bf16, kind="ExternalInput")
    WvAugPD = nc.dram_tensor("WvAugPD", [PAIRP, 2 * C], f8e4,
                             kind="ExternalInput")
    Wp = nc.dram_tensor("Wp", [C, C], bf16, kind="ExternalInput")
    masksBD = nc.dram_tensor("masksBD", [C, GROUPS * GROUPS], bf16,
                             kind="ExternalInput")
    bcMaskD = nc.dram_tensor("bcMaskD", [8, 1024], bf16, kind="ExternalInput")
    outT = nc.dram_tensor("outT", [C, QCH], f32, kind="ExternalOutput")
    # internal DRAM bounces (one tensor per checkpoint: whole-tensor deps)
    scRowD = [nc.dram_tensor(f"scRowD{j}", [HW], f8e4) for j in range(5)]
    shRowD = [nc.dram_tensor(f"shRowD{j}", [HW], f8e4) for j in range(5)]
    scRowQD = nc.dram_tensor("scRowQD", [QCH], bf16)
    rrD = [nc.dram_tensor(f"rrD{j}", [1024], bf16) for j in range(3)]

    with tile.TileContext(nc) as tc:
        import contextlib

        with contextlib.ExitStack() as ctx:
            consts = ctx.enter_context(tc.tile_pool(name="consts", bufs=1))
            big = ctx.enter_context(tc.tile_pool(name="big", bufs=1))
            # ALL psum comes from these two pools; disjoint regions: strips +
            # prelude/postlude share one 3-deep tag (6 banks), oT (2 banks).
            sps = ctx.enter_context(tc.tile_pool(name="sps", bufs=3, space="PSUM"))
            ops = ctx.enter_context(tc.tile_pool(name="ot_ps", bufs=1, space="PSUM"))
            sqp = ctx.enter_context(tc.tile_pool(name="sq_sb", bufs=2))
            stb = ctx.enter_context(tc.tile_pool(name="stat_sb", bufs=2))
            esb = ctx.enter_context(tc.tile_pool(name="exp_sb", bufs=6))
            osb = ctx.enter_context(tc.tile_pool(name="post_sb", bufs=2))

            # ---- big SBUF tensors (declared early; loads get top priority) --
            xb16 = big.tile([C, HW], bf16)       # raw bf16 x (batch stats)
            xq16 = big.tile([C, QCH], bf16)
            xqT_s = big.tile([C, QCH], f32)      # fp32 x kept only for residual
            x8p = big.tile([PAIRP, 2 * HW], f8e4)   # raw fp8 x, channel pairs
            xsb8 = big.tile([PAIRP, 2 * HW], f8e4)  # normalized fp8 pairs
            xsq = big.tile([CA, QCH], bf16)      # normalized local query chunk
            xsq_ones = xsq[C : C + 2, :]
            ones96b = consts.tile([VA, C], bf16)
            bcMask = consts.tile([8, 1024], bf16)

            wf_t = consts.tile([CA, CA], bf16)
            wvap = consts.tile([PAIRP, 2 * C], f8e4)
            wp_t = consts.tile([C, C], bf16)
            masksB = consts.tile([C, GROUPS * GROUPS], bf16)

            gt_b = consts.tile([GROUPS, GSPAN], f32, name="grow_B")
            bt_b = consts.tile([GROUPS, GSPAN], f32, name="brow_B")

            # transfers serialize per issuing queue; order each queue by when
            # its payloads unblock compute. x8p chunk i covers key tiles
            # 9i..9i+8 in BOTH halves via a 3d AP.
            def chunk(dst, src, i):
                sl = slice(i * CHK, (i + 1) * CHK)
                return dst[:, sl], src[:, sl]

            def x8chunk(i):
                dst = x8p[:, :].rearrange("p (i n) -> p i n", i=2)[
                    :, :, i * CHK : (i + 1) * CHK
                ]
                src = bass.AP(
                    tensor=x8pD, offset=i * CHK,
                    ap=[[2 * HW, PAIRP], [HW, 2], [1, CHK]],
                )
                return dst, src

            # Only the ramp-critical minimum is loaded up front; bulk loads
            # are emitted inside the prelude loops so the per-queue FIFOs
            # keep the stats->scale-row->broadcast chains near their heads.
            for eng, dst, src in [
                (nc.sync, *chunk(xq16, xqT16, 0)),
                (nc.sync, masksB, masksBD[:, :]),
                (nc.sync, *chunk(xq16, xqT16, 1)),
                (nc.sync, wf_t, WfoldD[:, :]),
                (nc.sync, gt_b, gRow[:, :]),
                (nc.sync, bt_b, bRow[:, :]),
                (nc.scalar, *chunk(xb16, xbT16, 0)),
                (nc.scalar, *chunk(xb16, xbT16, 1)),
                (None, None, None),  # ones-row memset slot (see below)
                (nc.scalar, *chunk(xb16, xbT16, 2)),
                (nc.scalar, *chunk(xb16, xbT16, 3)),
                (nc.sync, *x8chunk(0)),
                (nc.scalar, *x8chunk(2)),
                (nc.scalar, *x8chunk(3)),
            ]:
                if eng is None:
                    # xsq aug rows: ones; shift-row DMA overwrites row 96.
                    # vaug denominator column: constant ones at col 96 of
                    # every key tile (strided, 72 elements total).
                    nc.gpsimd.memset(xsq_ones, 1.0)
                    nc.gpsimd.memset(ones96b[C : C + 1, :], 1.0)
                    continue
                eng.dma_start(out=dst, in_=src)

            # dummy exp to trigger the ACT table load while DMAs run
            dumm = consts.tile([1, 8], f32)
            nc.vector.memset(dumm, 0.0)
            dumm2 = consts.tile([1, 8], f32)
            nc.scalar.activation(dumm2, dumm, AF.Exp)

            # ---- big SBUF tensors ----
            qT2p = big.tile([PAIRP, 2 * QCH], f8e4)  # fp8 folded q, pair halves
            vaug = big.tile([128, NTILES * VPAD], f8e4)
            oTr = big.tile([VA, QCH], bf16)      # attn@v evac; row 96 = rowsum
            rrB = big.tile([VA, QCH], bf16)      # 1/rowsum staged on part 96

            nc.gpsimd.memset(
                vaug[:, :].rearrange("p (t m) -> p t m", m=VPAD)[:, :, C : C + 1],
                1.0,
            )

            x8v = x8p[:, :].rearrange("p (i n) -> p i n", i=2)
            xs8v = xsb8[:, :].rearrange("p (i n) -> p i n", i=2)
            qT2v = qT2p[:, :].rearrange("p (i n) -> p i n", i=2)
            wvav = wvap[:, :].rearrange("p (i n) -> p i n", i=2)

            CNT = 1.0 / (GSPAN * C)
            SUBW = GSPAN // 4

            stats_ts = {}

            def stats_chunk(x16, key, i, first, last):
                """Colsums of groups 4i..4i+3 -> rows 4i+j of the side's
                persistent stats psum tile (lane-aligned)."""
                masks, ng = masksB, GROUPS
                if first:
                    stats_ts[key] = ops.tile([VA, 1024], f32, tag="oT",
                                             name="ts")
                ts_ = stats_ts[key]
                chunk = x16[:, i * CHK : (i + 1) * CHK]
                sq = sqp.tile([C, CHK], bf16, tag="sq", name="sq")
                # bf16 squares hit the DVE 2x_1p mode (660ns vs 960 Pool);
                # early batch chunks ride DVE's prelude idle, late ones go
                # to Pool so DVE exps aren't delayed.
                sq_eng = nc.vector if (key == "L" or i < 2) else nc.gpsimd
                sq_eng.tensor_mul(sq, chunk, chunk)
                ps_s = ts_[0:ng, 0:SUBW]
                ps_q = ts_[0:ng, 512 : 512 + SUBW]
                for j in range(4):
                    g = 4 * i + j
                    mk = masks[:, g * ng : (g + 1) * ng]
                    for u in range(4):
                        sspan = slice(
                            j * GSPAN + u * SUBW, j * GSPAN + (u + 1) * SUBW
                        )
                        st_ = first and (j == 0 and u == 0)
                        sp_ = last and (j == 3 and u == 3)
                        nc.tensor.matmul(
                            ps_s, mk, chunk[:, sspan], start=st_, stop=sp_,
                            skip_group_check=True,
                        )
                        nc.tensor.matmul(
                            ps_q, mk, sq[:, sspan], start=st_, stop=sp_,
                            skip_group_check=True,
                        )

            fin_prev = {"L": 0, "B": 0}

            def stats_core(key, k, ve=None):
                """Reduce the running sums for groups 0:k and compute
                mean/var/rstd (Quake seed + 1 Newton step). ve picks the
                scalar-chain engine: DVE for ramp-critical checkpoints,
                Pool for late ones (DVE is exp-saturated by then)."""
                ve = ve or nc.vector
                acc = stb.tile([GROUPS, 2], f32, tag="red", name="red")[:k]
                both = stats_ts[key][0:k, :].rearrange(
                    "p (a s) -> p a s", a=2
                )[:, :, 0:SUBW]
                nc.vector.tensor_reduce(acc, both, axis=AX.X, op=ALU.add)
                st = stb.tile([GROUPS, 12], f32, tag="st", name="st")[:k]
                mean, ex2 = st[:, 0:1], st[:, 1:2]
                msq, var = st[:, 2:3], st[:, 3:4]
                veps, ti = st[:, 4:5], st[:, 5:6]
                ya, yb = st[:, 6:7], st[:, 7:8]
                rstd = st[:, 8:9]
                ve.tensor_scalar_mul(mean, in0=acc[:, 0:1], scalar1=CNT)
                ve.tensor_scalar_mul(ex2, in0=acc[:, 1:2], scalar1=CNT)
                ve.tensor_mul(msq, mean, mean)
                ve.tensor_sub(var, ex2, msq)
                ve.tensor_scalar_add(veps, in0=var, scalar1=EPS)
                ve.tensor_scalar(
                    out=ti.bitcast(i32), in0=veps.bitcast(i32),
                    scalar1=1, scalar2=-1, op0=ALU.arith_shift_right,
                    op1=ALU.bitwise_xor,
                )
                ve.tensor_scalar_add(
                    rstd.bitcast(i32), in0=ti.bitcast(i32), scalar1=0x5F3759E0
                )
                ve.tensor_mul(ya, rstd, rstd)
                ve.tensor_mul(yb, ya, veps)
                ve.tensor_scalar(
                    out=yb, in0=yb, scalar1=-0.5, scalar2=1.5,
                    op0=ALU.mult, op1=ALU.add,
                )
                ve.tensor_mul(rstd, rstd, yb)
                return st

            def finish_B(j, k, ve=None):
                """Batch-side checkpoint: publish the fp8 scale row (pulled
                back per chunk as a pair broadcast) and the fp8 shift row
                (bounced into xsb8[48, h1]) for groups kp:k."""
                ve = ve or nc.vector
                st = stats_core("B", k, ve)
                kp = fin_prev["B"]
                fin_prev["B"] = k
                mean, rstd = st[:, 0:1], st[:, 8:9]
                sk = slice(kp, k)
                d = k - kp
                # DVE computes the f32 rows; SWDGE (gpsimd) DMAs cast
                # f32 -> fp8 while publishing, skipping a staging hop
                sc32 = stb.tile([GROUPS, GSPAN], bf16, tag="sc", name="sc32")
                ve.tensor_scalar_mul(
                    sc32[:k], in0=gt_b[:k], scalar1=rstd[:k]
                )
                ms32 = stb.tile([GROUPS, GSPAN], f32, tag="ms", name="ms32")
                ve.tensor_scalar_mul(
                    ms32[:k], in0=sc32[:k], scalar1=mean[:k]
                )
                sh32 = stb.tile([GROUPS, GSPAN], f32, tag="sh", name="sh32")
                ve.tensor_sub(sh32[:k], bt_b[:k], ms32[:k])
                nc.gpsimd.dma_start(
                    out=bass.AP(tensor=scRowD[j], offset=kp * GSPAN,
                                ap=[[GSPAN, d], [1, GSPAN]]),
                    in_=sc32[sk],
                )
                nc.gpsimd.dma_start(
                    out=bass.AP(tensor=shRowD[j], offset=kp * GSPAN,
                                ap=[[GSPAN, d], [1, GSPAN]]),
                    in_=sh32[sk],
                )
                if kp < QGROUPS:
                    # own-group rows double as the q-side scale/shift: the
                    # host rotated the batch chunks so this core's query
                    # groups are batch groups 0:8.
                    kq = min(k, QGROUPS)
                    nc.sync.dma_start(
                        out=scRowQD[kp * GSPAN : kq * GSPAN].rearrange(
                            "(g s) -> g s", s=GSPAN
                        ),
                        in_=sc32[kp:kq],
                    )
                    nc.gpsimd.dma_start(
                        out=xsq[C : C + 1, kp * GSPAN : kq * GSPAN].rearrange(
                            "p (g s) -> p g s", g=kq - kp
                        ),
                        in_=sh32[kp:kq],
                    )
                return sc32

            def shrow_bounce(j, kp, k):
                """Pull the published shift row into the aug channel (pair
                slot 48/h1). The scale multiplies never touch that row, so
                this bounce runs concurrently with them."""
                nc.sync.dma_start(
                    out=xsb8[PAIRP - 1 : PAIRP,
                             HW + kp * GSPAN : HW + k * GSPAN],
                    in_=bass.AP(tensor=shRowD[j], offset=kp * GSPAN,
                                ap=[[HW, 1], [1, (k - kp) * GSPAN]]),
                )

            def bcast_row_ps(rows, k0, nk, npart):
                """PE broadcast of per-group scale rows into PSUM: two
                sps tiles, one group per 512-aligned slot (matmul outs may
                not cross psum banks). Returns per-group [npart, 288]
                views."""
                ta = sps.tile([128, 1024], f32, tag="sp", name="bcA")
                tb = sps.tile([128, 1024], f32, tag="sp", name="bcB")
                # groups 0-2 contiguous in ta (sub-matmuls split at the
                # psum bank boundary; the TT reads span banks freely),
                # group 3 in tb.
                for g in range(nk):
                    ga = k0 + g
                    spans = ([(g * GSPAN, GSPAN)] if g in (0, 3)
                             else ([(288, 224), (512, 64)] if g == 1
                                   else [(576, 288)]))
                    t_ = tb if g == 3 else ta
                    off = 0
                    for o0, w in spans:
                        dst = t_[0:npart, (0 if g == 3 else o0)
                                 : (0 if g == 3 else o0) + w]
                        nc.tensor.matmul(
                            dst,
                            bcMask[0 : k0 + 4, ga * 128 : ga * 128 + npart],
                            rows[0 : k0 + 4, off : off + w],
                            start=True, stop=True,
                        )
                        off += w
                return ta, tb

            def scaled_chunk_ps(i, rows):
                """Ramp-critical chunks: scale via the PE-broadcast PSUM
                row (DVE TT with one PSUM operand) - no DRAM roundtrip."""
                ta, tb = bcast_row_ps(rows, 4 * i, 4, PAIRP)
                lo = i * CHK
                for (o0, w), t_ in [((0, 864), ta), ((864, GSPAN), tb)]:
                    nc.vector.tensor_mul(
                        xs8v[:, 0, lo + o0 : lo + o0 + w],
                        x8v[:, 0, lo + o0 : lo + o0 + w],
                        t_[0:PAIRP, 0:w],
                    )
                    nc.vector.tensor_mul(
                        xs8v[0 : PAIRP - 1, 1, lo + o0 : lo + o0 + w],
                        x8v[0 : PAIRP - 1, 1, lo + o0 : lo + o0 + w],
                        t_[0 : PAIRP - 1, 0:w],
                    )

            def scaled_chunk_B(i, j, eng=None):
                """xsb8 = x8p * scale for chunk i (both halves in one op);
                scale comes back as a 49-partition x 2-half broadcast of
                the published fp8 row. Default Pool; the first chunks ride
                DVE's prelude idle to shorten the Pool serial chain."""
                scb = sqp.tile([PAIRP, 2 * CHK], f8e4, tag="scb", name="scb")
                bcast_src = bass.AP(
                    tensor=scRowD[j], offset=i * CHK,
                    ap=[[0, PAIRP], [0, 2], [1, CHK]],
                )
                nc.sync.dma_start(
                    out=scb[:, :].rearrange("p (i n) -> p i n", i=2),
                    in_=bcast_src,
                )
                e_ = eng or nc.gpsimd
                e_.tensor_mul(
                    xs8v[:, 0, i * CHK : (i + 1) * CHK],
                    x8v[:, 0, i * CHK : (i + 1) * CHK],
                    scb[:, 0:CHK],
                )
                e_.tensor_mul(
                    xs8v[0 : PAIRP - 1, 1, i * CHK : (i + 1) * CHK],
                    x8v[0 : PAIRP - 1, 1, i * CHK : (i + 1) * CHK],
                    scb[0 : PAIRP - 1, CHK : 2 * CHK],
                )

            def scaled_chunk_L(i, rows=None):
                """xsq = xq16 * scaleQ. Chunk 0 (ramp-critical) gets the
                scale row as a PE-broadcast PSUM tile; chunk 1 via the
                partition-broadcast DMA on Pool."""
                sl = slice(i * CHK, (i + 1) * CHK)
                if rows is not None:
                    ta, tb = bcast_row_ps(rows, 4 * i, 4, C)
                    lo = i * CHK
                    for (o0, w), t_ in [((0, 864), ta), ((864, GSPAN), tb)]:
                        nc.vector.tensor_mul(
                            xsq[0:C, lo + o0 : lo + o0 + w],
                            xq16[:, lo + o0 : lo + o0 + w],
                            t_[0:C, 0:w],
                        )
                    return
                scb = sqp.tile([C, CHK], bf16, tag="scbL", name="scbL")
                bcast_src = bass.AP(
                    tensor=scRowQD, offset=i * CHK, ap=[[0, C], [1, CHK]]
                )
                nc.sync.dma_start(out=scb, in_=bcast_src)
                nc.gpsimd.tensor_mul(xsq[0:C, sl], xq16[:, sl], scb)

            def emit_vaug(i):
                """v-projection for chunk i's 9 key tiles: fp8 DoubleRow
                matmuls from the normalized pair tensor; plain fp8 evacs
                write cols 0:96 (col 96 is the constant ones column)."""
                t0 = i * 9
                for base, cnt in [(0, 5), (5, 4)]:
                    tv = sps.tile([128, 512], f32, tag="sp", name="tv")
                    for jj in range(cnt):
                        t = t0 + base + jj
                        nc.tensor.matmul(
                            tv[:, jj * C : (jj + 1) * C],
                            xs8v[:, :, t * 128 : (t + 1) * 128],
                            wvav,
                            start=True, stop=True, perf_mode=DR,
                        )
                    dst = vaug[
                        :, (t0 + base) * VPAD : (t0 + base + cnt) * VPAD
                    ].rearrange("p (t m) -> p t m", t=cnt)[:, :, 0:C]
                    srcv = tv[:, 0 : cnt * C].rearrange(
                        "p (t m) -> p t m", t=cnt
                    )
                    if cnt == 5:
                        nc.scalar.activation(dst, srcv, AF.Copy)
                    else:
                        # DVE picks up the smaller evac: ACT is the
                        # busy-bound engine in steady state
                        nc.vector.tensor_copy(dst, srcv)

            def qside_chunk(i):
                """qT2p spans for q-chunk i: scale multiply then two
                half-column matmuls per 512-span into one [49, 1024] psum,
                one ACT evac each. Gated on the batch checkpoint covering
                this core's own groups (host-rotated to the front)."""
                scaled_chunk_L(i)
                for off in (0, 512, 1024):
                    lo = i * CHK + off
                    w = min(512, (i + 1) * CHK - lo)
                    tq = sps.tile([128, 1024], f32, tag="sp", name="tq")
                    for half in range(2):
                        nc.tensor.matmul(
                            tq[0:PAIRP, half * 512 : half * 512 + w],
                            wf_t[:, half * PAIRP : (half + 1) * PAIRP],
                            xsq[:, lo : lo + w],
                            start=True, stop=True,
                        )
                    nc.scalar.activation(
                        qT2v[:, :, lo : lo + w],
                        tq[0:PAIRP, :].rearrange("p (i n) -> p i n", i=2)[
                            :, :, 0:w
                        ],
                        AF.Copy,
                    )

            # ---- batch prelude: stats -> scale rows -> xsb8 per chunk.
            # Four checkpoints so the tail chunks' scaling isn't gated on
            # the full-batch stats; the x8p tail loads are emitted AFTER
            # each checkpoint's critical scale-row broadcasts (queue FIFO).
            for i in range(4):
                stats_chunk(xb16, "B", i, first=(i == 0), last=False)
                if i == 0:
                    finish_B(0, 4)
                    qside_chunk(0)
                    scaled_chunk_B(0, 0, nc.vector)
                    shrow_bounce(0, 0, 4)
                    for eng, dst, src in [
                        (nc.sync, *x8chunk(1)),
                        (nc.scalar, *chunk(xb16, xbT16, 4)),
                        (nc.scalar, *chunk(xb16, xbT16, 5)),
                    ]:
                        eng.dma_start(out=dst, in_=src)
                elif i == 1:
                    for eng, dst, src in [
                        (nc.sync, wvap, WvAugPD[:, :]),
                        (nc.sync, bcMask, bcMaskD[:, :]),
                        (nc.sync, *chunk(xb16, xbT16, 6)),
                        (nc.sync, *chunk(xb16, xbT16, 7)),
                    ]:
                        eng.dma_start(out=dst, in_=src)
                    emit_vaug(0)
                elif i == 3:
                    finish_B(2, 16)
                    qside_chunk(1)
                    scaled_chunk_B(1, 2, nc.vector)
                    shrow_bounce(2, 4, 8)
                    emit_vaug(1)
                    for eng, dst, src in [
                        (nc.sync, *x8chunk(4)),
                        (nc.sync, *x8chunk(5)),
                        (nc.sync, *x8chunk(6)),
                        (nc.sync, *x8chunk(7)),
                    ]:
                        eng.dma_start(out=dst, in_=src)

            # postlude-only loads, emitted here so they sit behind the
            # stats chain in the gpsimd queue
            for dst, src_ in [
                (xqT_s[:, 0:CHK], xqT[:, 0:CHK]),
                (xqT_s[:, CHK : 2 * CHK], xqT[:, CHK : 2 * CHK]),
                (wp_t, Wp[:, :]),
            ]:
                nc.gpsimd.dma_start(out=dst, in_=src_)

            mb_count = [0]

            def mb_open(mw):
                pattern = EXP_PATTERN_MB[mb_count[0]]
                mb_count[0] += 1
                return {
                    "oT": ops.tile([VA, 1024], f32, tag="oT", name="oT"),
                    "pend": [], "next": 0,
                    "spb": 1024 // mw,
                    "halves": [(h, min(512, mw - h)) for h in range(0, mw, 512)],
                    "ex_half": [None],
                    "pattern": pattern, "ctr": [0],
                }

            def mb_emit(st, mo, mw, upto_tile):
                spb, halves = st["spb"], st["halves"]
                nst = NTILES // spb
                while st["next"] < nst and st["next"] * spb < upto_tile:
                    s = st["next"]
                    sp = sps.tile([128, 1024], f32, tag="sp", name="sp")
                    for j in range(spb):
                        t = s * spb + j
                        for h, hw_ in halves:
                            nc.tensor.matmul(
                                sp[:, j * mw + h : j * mw + h + hw_],
                                xs8v[:, :, t * 128 : (t + 1) * 128],
                                qT2v[:, :, mo + h : mo + h + hw_],
                                start=True, stop=True, perf_mode=DR,
                            )
                    c = st["ctr"][0]
                    st["ctr"][0] += 1
                    eng_c = st["pattern"][c]
                    if spb == 1:
                        # pair tile [128, 2048]: halves = strips 2P, 2P+1
                        if st["ex_half"][0] is None:
                            ex = esb.tile([128, 2048], f8e4, tag="ex", name="ex")
                            st["ex_half"][0] = ex
                            dst = ex[:, 0:1024]
                            pair_done = False
                        else:
                            ex = st["ex_half"][0]
                            dst = ex[:, 1024:2048]
                            st["ex_half"][0] = None
                            pair_done = True
                    else:
                        ex = esb.tile([128, 1024], f8e4, tag="ex", name="ex")
                        dst = ex[:, 0:1024]
                        pair_done = True
                    if eng_c == "A":
                        nc.scalar.activation(
                            dst, sp[:, : spb * mw], AF.Exp, scale=SCALE
                        )
                    else:
                        nc.vector.tensor_scalar(
                            out=dst.bitcast(i8), in0=sp[:, : spb * mw],
                            scalar1=A_DVE, scalar2=B_DVE,
                            op0=ALU.mult, op1=ALU.add,
                        )
                    if pair_done:
                        st["pend"].append((s, ex))
                        if len(st["pend"]) > 3:
                            _mb_c(st, mo, mw)
                    st["next"] += 1

            def _mb_c(st, mo, mw):
                """DoubleRow attn@V for one ready pair group: contract 256
                keys per matmul (2 key-tiles in the free-dim pair axis)."""
                spb, halves = st["spb"], st["halves"]
                s_, ex_ = st["pend"].pop(0)
                if spb == 1:
                    P = s_ // 2
                    va = vaug[:, 2 * P * VPAD : (2 * P + 2) * VPAD].rearrange(
                        "p (i m) -> p i m", i=2
                    )[:, :, 0:VA]
                    exp_pair = ex_[:, :].rearrange("p (i n) -> p i n", i=2)
                    for h, hw_ in halves:
                        nc.tensor.matmul(
                            st["oT"][:, h : h + hw_],
                            va, exp_pair[:, :, h : h + hw_],
                            start=(P == 0), stop=(P == NPAIRS - 1),
                            perf_mode=DR,
                        )
                else:
                    for j in range(2):
                        P = 2 * s_ + j
                        va = vaug[:, 2 * P * VPAD : (2 * P + 2) * VPAD].rearrange(
                            "p (i m) -> p i m", i=2
                        )[:, :, 0:VA]
                        exp_pair = ex_[:, j * 2 * mw : (j + 1) * 2 * mw].rearrange(
                            "p (i n) -> p i n", i=2
                        )
                        nc.tensor.matmul(
                            st["oT"][:, 0:mw],
                            va, exp_pair,
                            start=(P == 0), stop=(P == NPAIRS - 1),
                            perf_mode=DR,
                        )

            mb_idx = [0]

            def mb_finish_head(st, mo, mw):
                """Drain the pair backlog and start the postlude chain
                (evac -> recip -> 1/r publish)."""
                while st["pend"]:
                    _mb_c(st, mo, mw)
                oT = st["oT"]
                rd = rrD[min(mb_idx[0], 2)]
                mb_idx[0] += 1
                final = mb_idx[0] == 3
                st["fin"] = (rd, final)
                nc.vector.tensor_copy(oTr[:, mo : mo + mw], oT[0:VA, :mw])
                with nc.allow_low_precision(reason="softmax denom; bf16 ok"):
                    nc.vector.reciprocal(
                        rrB[C : C + 1, mo : mo + mw],
                        oTr[C : C + 1, mo : mo + mw],
                    )
                if not final:
                    nc.sync.dma_start(
                        out=rd[0:mw].rearrange("(a n) -> a n", a=1),
                        in_=rrB[C : C + 1, mo : mo + mw],
                    )

            def mb_finish_body(st, mo, mw):
                rd, final = st["fin"]
                po = mo
                while po < mo + mw:
                    pw = min(512, mo + mw - po)
                    tc_ = sps.tile([128, 512], f32, tag="sp", name="tpost")
                    pp = tc_[0:C, 0:pw]
                    nc.tensor.matmul(
                        pp, wp_t, oTr[0:C, po : po + pw], start=True, stop=True
                    )
                    sc = osb.tile([C, 512], f32, tag="sc", name="sc")
                    if final:
                        tc2 = sps.tile([128, 512], f32, tag="sp", name="tpo2")
                        pr = tc2[0:C, 0:pw]
                        nc.tensor.matmul(
                            pr, ones96b[C : C + 1, :],
                            rrB[C : C + 1, po : po + pw],
                            start=True, stop=True, tile_position=(96, 0),
                        )
                        prs = osb.tile([C, 512], bf16, tag="rbc", name="prs")
                        nc.vector.tensor_copy(prs[:, :pw], pr)
                        nc.vector.tensor_mul(sc[:, :pw], pp, prs[:, :pw])
                    else:
                        rbc = osb.tile([C, 512], bf16, tag="rbc", name="rbc")
                        nc.sync.dma_start(
                            out=rbc[:, :pw],
                            in_=bass.AP(tensor=rd, offset=po - mo,
                                        ap=[[0, C], [1, pw]]),
                        )
                        nc.vector.tensor_mul(sc[:, :pw], pp, rbc[:, :pw])
                    ot = osb.tile([C, 512], f32, tag="ot", name="ot")
                    add_eng = nc.vector if final else nc.gpsimd
                    add_eng.tensor_add(
                        ot[:, :pw], sc[:, :pw], xqT_s[:, po : po + pw]
                    )
                    nc.sync.dma_start(out=outT[:, po : po + pw], in_=ot[:, :pw])
                    po += pw

            def mb_finish(st, mo, mw):
                mb_finish_head(st, mo, mw)
                mb_finish_body(st, mo, mw)

            # tail batch stats + the last checkpoint interleave with the
            # mb0 strips (their DVE/PE work would otherwise sit at the
            # engine FIFO heads blocking the first exps)
            st0 = mb_open(1024)
            for i in range(8):
                if i == 0:
                    stats_chunk(xb16, "B", 4, first=False, last=False)
                    scaled_chunk_B(2, 2)
                    scaled_chunk_B(3, 2)
                    shrow_bounce(2, 8, 16)
                elif i == 1:
                    stats_chunk(xb16, "B", 5, first=False, last=False)
                elif i == 2:
                    stats_chunk(xb16, "B", 6, first=False, last=False)
                    emit_vaug(2)
                elif i == 3:
                    stats_chunk(xb16, "B", 7, first=False, last=True)
                    emit_vaug(3)
                elif i == 4:
                    finish_B(4, GROUPS)
                    scaled_chunk_B(4, 4)
                    scaled_chunk_B(5, 4)
                    scaled_chunk_B(6, 4)
                    scaled_chunk_B(7, 4)
                    shrow_bounce(4, 16, GROUPS)
                    emit_vaug(4)
                elif i >= 5:
                    emit_vaug(i)
                mb_emit(st0, 0, 1024, 9 * (i + 1))
            # bridge m-block boundaries: pre-emit the next block's first
            # strips before draining this block's tail so ACT/DVE never
            # idle across the transition.
            st1 = mb_open(1024)
            mb_emit(st1, 1024, 1024, 4)
            mb_finish_head(st0, 0, 1024)
            mb_emit(st1, 1024, 1024, 28)
            mb_finish_body(st0, 0, 1024)
            mb_emit(st1, 1024, 1024, NTILES)
            st2 = mb_open(256)
            mb_emit(st2, 2048, 256, 8)
            mb_finish_head(st1, 1024, 1024)
            mb_emit(st2, 2048, 256, NTILES)
            mb_finish_body(st1, 1024, 1024)
            mb_finish(st2, 2048, 256)

    _split_multiwaits(nc)
    return nc


def _prep_inputs(x, gamma, beta, Wq, bq, Wk, bk, Wv, bv, Wp, bp):
    bf16 = ml_dtypes.bfloat16
    f8 = ml_dtypes.float8_e4m3fn if hasattr(ml_dtypes, "float8_e4m3fn") \
        else ml_dtypes.float8_e4m3
    f32 = np.float32

    x2 = np.ascontiguousarray(x.reshape(B, HW, C))
    gam = np.repeat(np.asarray(gamma, np.float64), W)      # [HW] per position
    bet = np.repeat(np.asarray(beta, np.float64), W)
    gRow = gam.reshape(GROUPS, GSPAN).astype(f32)
    bRow = bet.reshape(GROUPS, GSPAN).astype(f32)

    # WvAug: aug channels (96 Wv rows + shift-coeff row + zero pad), v cols
    # only - bv folds into the residual, the denominator is a vaug ones col.
    WvAug = np.zeros((CA, C), f32)
    WvAug[:C, :] = Wv
    WvAug[C, :] = Wv.sum(axis=0)       # u_v: shift-row coefficient
    WvAugP = np.zeros((PAIRP, 2 * C), f32)
    WvAugP[:, 0:C] = WvAug[0:PAIRP]
    WvAugP[0 : C - PAIRP, C : 2 * C] = WvAug[PAIRP:C]
    WvAugP[PAIRP - 1, C : 2 * C] = WvAug[C]

    def aug(Wm, bias):
        a = np.empty((CA, C), f32)
        a[:C] = Wm
        a[C] = Wm.sum(axis=0)
        a[C + 1] = bias
        return a

    WqAug = aug(np.asarray(Wq, f32), bq)
    WkAugT = np.ascontiguousarray(aug(np.asarray(Wk, f32), bk).T)
    Wfold = (WqAug.astype(np.float64) @ WkAugT.astype(np.float64)).astype(f32)
    # k-side slot permutation: pair slot (47,h1) = zero pad, (48,h1) = the
    # shift channel (so scale multiplies never touch the shift row); the
    # k-side bias channel is softmax-invariant and dropped entirely.
    Wf2 = np.zeros_like(Wfold)
    Wf2[:, 0:C] = Wfold[:, 0:C]
    Wf2[:, C + 1] = Wfold[:, C]
    Wfold = Wf2

    masksB = np.zeros((C, GROUPS * GROUPS), f32)
    for g in range(GROUPS):
        masksB[:, g * GROUPS + g] = 1.0
    masksL = np.zeros((C, QGROUPS * QGROUPS), f32)
    for g in range(QGROUPS):
        masksL[:, g * QGROUPS + g] = 1.0
    bcMask = np.zeros((8, 1024), f32)
    for g in range(8):
        bcMask[g, g * 128 : (g + 1) * 128] = 1.0

    # residual with the output-projection bias AND bv@Wp pre-added (bv is
    # attention-invariant: softmax weights sum to 1)
    radd = (np.asarray(bp, np.float64)
            + np.asarray(bv, np.float64) @ np.asarray(Wp, np.float64))

    in_maps = []
    for core in range(NCORES):
        b, qc = divmod(core, 4)
        xbT = np.ascontiguousarray(x2[b].T)
        xqT = np.ascontiguousarray(xbT[:, qc * QCH : (qc + 1) * QCH])
        # rotate the batch chunk order so this core's own query chunks come
        # first: its GroupNorm groups are then batch groups 0:8, and the
        # first batch stats checkpoints double as the q-side stats (the
        # separate q-stats pipeline is deleted). Key order is irrelevant to
        # the attention math as long as every k-side tensor agrees.
        perm = [2 * qc, 2 * qc + 1] + [c for c in range(8)
                                       if c not in (2 * qc, 2 * qc + 1)]
        cperm = np.concatenate([np.arange(c * CHK, (c + 1) * CHK)
                                for c in perm])
        xbTr = np.ascontiguousarray(xbT[:, cperm])
        gamr = gam[cperm]
        betr = bet[cperm]
        x8 = xbTr.astype(f8)
        x8pD = np.zeros((PAIRP, 2 * HW), f8)
        x8pD[:, 0:HW] = x8[0:PAIRP]
        x8pD[0 : C - PAIRP, HW : 2 * HW] = x8[PAIRP:C]
        in_maps.append({
            "xbT16": xbTr.astype(bf16),
            "x8pD": x8pD,
            "xqT16": xqT.astype(bf16),
            "xqT": (xqT.astype(np.float64)
                    + radd.reshape(C, 1)).astype(f32),
            "gRow": gamr.reshape(GROUPS, GSPAN).astype(f32),
            "bRow": betr.reshape(GROUPS, GSPAN).astype(f32),
            "WfoldD": Wfold.astype(bf16),
            "WvAugPD": WvAugP.astype(f8),
            "Wp": np.asarray(Wp, f32).astype(bf16),
            "masksBD": masksB.astype(bf16),
            "bcMaskD": bcMask.astype(bf16),
        })
    return in_maps


def _get_sharded_fn():
    """Build the 8-core shard_map callable once so repeated calls reuse the
    compiled NEFF executable."""
    if "fn" in _compiled:
        return _compiled["fn"]

    import jax
    from jax.sharding import Mesh, PartitionSpec
    from jax.experimental.shard_map import shard_map
    import concourse.mybir as mybir
    from concourse.bass2jax import (
        _bass_exec_p, install_neuronx_cc_hook, partition_id_tensor
    )

    if "nc" not in _compiled:
        _compiled["nc"] = _build_bass()
    nc = _compiled["nc"]
    install_neuronx_cc_hook()

    pname = nc.partition_id_tensor.name if nc.partition_id_tensor else None
    in_names, out_names, out_avals = [], [], []
    for alloc in nc.m.functions[0].allocations:
        if not isinstance(alloc, mybir.MemoryLocationSet):
            continue
        name = alloc.memorylocations[0].name
        if alloc.kind == "ExternalInput":
            if name != pname:
                in_names.append(name)
        elif alloc.kind == "ExternalOutput":
            out_names.append(name)
            out_avals.append(
                jax.core.ShapedArray(
                    tuple(alloc.tensor_shape), mybir.dt.np(alloc.dtype)
                )
            )
    n_params = len(in_names)
    all_names = in_names + out_names
    if pname is not None:
        all_names = all_names + [pname]

    def _body(*args):
        operands = list(args)
        if pname is not None:
            operands.append(partition_id_tensor())
        outs = _bass_exec_p.bind(
            *operands,
            out_avals=tuple(out_avals),
            in_names=tuple(all_names),
            out_names=tuple(out_names),
            lowering_input_output_aliases=(),
            sim_require_finite=True,
            sim_require_nnan=True,
            nc=nc,
        )
        return tuple(outs)

    devices = jax.devices()[:NCORES]
    mesh = Mesh(np.asarray(devices), ("core",))
    sharded = jax.jit(
        shard_map(
            _body, mesh=mesh,
            in_specs=(PartitionSpec("core"),) * (n_params + len(out_names)),
            out_specs=(PartitionSpec("core"),) * len(out_names),
            check_rep=False,
        ),
        keep_unused=True,
    )

    from jax.sharding import NamedSharding

    shard = NamedSharding(mesh, PartitionSpec("core"))

    def put(in_maps):
        """Upload per-core inputs + zero outputs once; reuse across calls."""
        dev = [
            jax.device_put(
                np.concatenate(
                    [np.asarray(in_maps[c][nm]) for c in range(NCORES)], axis=0
                ),
                shard,
            )
            for nm in in_names
        ]
        dev += [
            jax.device_put(
                np.zeros((NCORES * a.shape[0], *a.shape[1:]), a.dtype), shard
            )
            for a in out_avals
        ]
        return dev

    def execute(dev_in):
        return sharded(*dev_in)

    def run(in_maps):
        out_arrs = execute(put(in_maps))
        return {
            nm: np.asarray(out_arrs[i]).reshape(NCORES, *out_avals[i].shape)
            for i, nm in enumerate(out_names)
        }

    _compiled["fn"] = (run, out_names, put, execute)
    _compiled["mkchain"] = (sharded, in_names, out_names, _body)
    return _compiled["fn"]


def kernel(x, gamma, beta, Wq, bq, Wk, bk, Wv, bv, Wp, bp):
    run = _get_sharded_fn()[0]
    in_maps = _prep_inputs(
        np.asarray(x, np.float32), gamma, beta, Wq, bq, Wk, bk, Wv, bv, Wp, bp
    )
    res = run(in_maps)["outT"]

    out = np.empty((B, HW, C), np.float32)
    for core in range(NCORES):
        b, qc = divmod(core, 4)
        out[b, qc * QCH : (qc + 1) * QCH, :] = res[core].T
    return out.reshape(B, H, W, C)
